# revision 33
# baseline (speedup 1.0000x reference)
"""Trainium2 Bass kernel for nn_Attention (dense transformer attention w/ QK-LayerNorm).

Sharding: sequence-parallel over 8 cores. Core c handles batch b = c//2,
token half h = c%2 (512 tokens). K/V are computed redundantly for the full
batch element on both cores of a pair; Q only for local tokens. No
collectives.

All matmul operands are bf16 (fp32 PSUM accumulate); LN stats and softmax
denominators stay fp32. Dataflow is transpose-free: Q/K are produced
channel-major so the QK-LayerNorm reductions over C become ones-vector
matmuls on the partition axis, and S^T tiles feed softmax-exp directly; V is
produced token-major with an appended ones-column per head (66-wide slabs
for 4B alignment) so the PV matmul emits the softmax denominator as an extra
output row.

Scheduling keeps the PE queue dense (HAM stays at 2.4GHz): squares and
PSUM evictions run on ACT, the LN stats chain runs multi-partition on rows
0/32/64 during the V phase with its ones-matmul broadcasts interleaved
between V matmuls, K/Q normalization (DVE) overlaps the V matmuls, and the
per-head softmax-denominator reciprocal (DVE approx) + ones-matmul
broadcast + scale is deferred into the NEXT head-pair's PE slot so it never
blocks the in-order PE queue; the last pair's scale is woven into the
m-outer output projection, whose per-tile evict+DMA overlaps the next
tile's accumulation. PV stationary slabs are padded to 128 columns (fast
weight load); all big DMAs are host-pre-arranged dense [128, X] blocks.
"""

import numpy as np

B, N, C = 4, 1024, 1024
H, D = 16, 64
LN_EPS = 1e-5
N_CORES = 8
TL = 512          # local tokens per core
KT = 8            # channel tiles of 128
SCALE = D ** -0.5

_COMPILED = None


def _build():
    import concourse.bacc as bacc
    import concourse.tile as tile
    import concourse.mybir as mybir

    F32 = mybir.dt.float32
    BF = mybir.dt.bfloat16
    AF = mybir.ActivationFunctionType
    OP = mybir.AluOpType

    nc = bacc.Bacc("TRN2", target_bir_lowering=False, debug=False,
                   num_devices=N_CORES)

    xT_d = nc.dram_tensor("xT", [128, KT * N], BF, kind="ExternalInput").ap()
    wk_d = nc.dram_tensor("wk", [128, KT * C], BF, kind="ExternalInput").ap()
    wq_d = nc.dram_tensor("wq", [128, KT * C], BF, kind="ExternalInput").ap()
    wv_d = nc.dram_tensor("wv", [128, KT * C], BF, kind="ExternalInput").ap()
    wp_d = nc.dram_tensor("wp", [128, KT * C], BF, kind="ExternalInput").ap()
    wsums_d = nc.dram_tensor("wsums", [128, 16], BF, kind="ExternalInput").ap()
    params_d = nc.dram_tensor("params", [128, 56], F32, kind="ExternalInput").ap()
    bsum_d = nc.dram_tensor("bsum3", [128, 1], F32, kind="ExternalInput").ap()
    out_d = nc.dram_tensor("out", [C, TL], F32, kind="ExternalOutput").ap()

    with tile.TileContext(nc) as tc:
        with tc.tile_pool(name="persist", bufs=1) as pers, \
             tc.tile_pool(name="small", bufs=1) as smallp:

            khat = pers.tile([128, KT * N], BF, tag="khat")        # 2MB
            vful = pers.tile([128, KT * 16 * 66 + 128], BF, tag="vful")
            qhat = pers.tile([128, KT * TL], BF, tag="qhat")       # 1MB
            osb = pers.tile([128, KT * TL], BF, tag="osb")         # 1MB
            rstdb = pers.tile([128, 2 * N + 2 * TL], BF, tag="rstdb")
            wp = pers.tile([128, KT * C], BF, tag="wp")            # 2MB
            qzs = [pers.tile([128, 1024], BF, tag=f"qz{i}", name=f"qz{i}")
                   for i in range(2)]

            params = smallp.tile([128, 56], F32, tag="params")
            wsums = smallp.tile([128, 16], BF, tag="wsums")
            bsum = smallp.tile([128, 1], F32, tag="bsum")
            sel = smallp.tile([128, 4], BF, tag="sel")
            ones_big = smallp.tile([128, 128], BF, tag="ones_big")
            # stats rows: 0 = K tokens 0:512, 32 = K 512:1024, 64 = Q.
            # col blocks of 512: 0 sums, 1 ssq->var->ln(var), 2 mu, 3 mu2
            st = smallp.tile([128, 4 * 512], F32, tag="st")
            stmm = smallp.tile([128, 2 * 512], BF, tag="stmm")  # rstd, murstd
            dden = smallp.tile([64, 2048], F32, tag="dden")
            drecip = smallp.tile([64, 2048], F32, tag="drecip")

            def sl(i, w=512):
                return st[0:65, i * 512: i * 512 + w]

            def slr(row, i, w=512):
                return st[row:row + 1, i * 512: i * 512 + w]

            def smmr(row, i):
                return stmm[row:row + 1, i * 512:(i + 1) * 512]

            def prm(grp, kt):  # qn_w qn_b kn_w kn_b qb kb projb
                return params[:, grp * 8 + kt: grp * 8 + kt + 1]

            def kh(m):
                return khat[:, m * N:(m + 1) * N]

            def qh(m):
                return qhat[:, m * TL:(m + 1) * TL]

            def vf(mt, h):
                # 128-wide stationary (cols 65..127 are neighbor-slab
                # padding; they only feed unused PSUM rows 65..127) so the
                # PE can use fast weight load
                base = (mt * 16 + h) * 66
                return vful[:, base: base + 128]

            # ---- DMAs, ordered by consumption ----
            nc.sync.dma_start(wsums[:], wsums_d[:])

            with tc.tile_pool(name="xp", bufs=1) as xpool, \
                 tc.tile_pool(name="wpool", bufs=1) as wpool, \
                 tc.tile_pool(name="sq", bufs=2) as sqp, \
                 tc.tile_pool(name="ntmp", bufs=3) as ntp, \
                 tc.tile_pool(name="ps", bufs=8, space="PSUM") as ps:
                xT = xpool.tile([128, KT * N], BF, tag="xT")       # 2MB
                for hx in range(2):
                    nc.sync.dma_start(xT[:, hx * 4096:(hx + 1) * 4096],
                                      xT_d[:, hx * 4096:(hx + 1) * 4096])

                wk = wpool.tile([128, KT * C], BF, tag="wk")
                wq = wpool.tile([128, KT * C], BF, tag="wq")
                wv = wpool.tile([128, KT * C], BF, tag="wv")

                for hw in range(2):
                    nc.sync.dma_start(wk[:, hw * 4096:(hw + 1) * 4096],
                                      wk_d[:, hw * 4096:(hw + 1) * 4096])
                nc.sync.dma_start(params[:], params_d[:])
                nc.sync.dma_start(bsum[:], bsum_d[:])
                nc.sync.dma_start(wq[:], wq_d[:])
                nc.sync.dma_start(wv[:], wv_d[:])
                nc.sync.dma_start(wp[:], wp_d[:])

                nc.vector.memset(sel[:], 0.0)
                nc.vector.memset(sel[:, 0:1], 1.0)   # ones column (reduce)
                nc.vector.memset(ones_big[:], 1.0)
                nc.vector.memset(st[:], 0.0)
                vv = vful[:, 0:KT * 16 * 66].rearrange(
                    "p (a h e) -> p a h e", a=KT, h=16)
                nc.vector.memset(vv[:, :, :, 65:66], 0.0)
                nc.vector.memset(vful[:, KT * 16 * 66:], 0.0)
                for q in qzs:
                    nc.vector.memset(q[:], 0.0)
                nc.vector.memset(dden[:], 1.0)
                for mt in range(KT):
                    nc.vector.memset(vv[:, mt, :, 64:65], 1.0)

                def xsl(kt, c0, w):
                    return xT[:, kt * N + c0: kt * N + c0 + w]

                def wsl(t, kt, c0, w):
                    return t[:, kt * C + c0: kt * C + c0 + w]

                # ---- token sums of (q,k): row-0 accumulators ----
                sums_ps = [ps.tile([1, 512], F32, tag="ps", name=f"sums{i}")
                           for i in range(3)]  # Knh0, Knh1, Q
                for kt in range(KT):
                    for nh in range(2):
                        nc.tensor.matmul(
                            sums_ps[nh][:],
                            wsums[:, kt * 2 + 1: kt * 2 + 2],
                            xsl(kt, nh * 512, 512),
                            start=(kt == 0), stop=(kt == KT - 1))
                    nc.tensor.matmul(
                        sums_ps[2][:], wsums[:, kt * 2: kt * 2 + 1],
                        xsl(kt, 0, 512),
                        start=(kt == 0), stop=(kt == KT - 1))

                # ---- K phase; eviction + square on ACT; ssq reductions
                #      (PE) trail one m behind the squares ----
                ssqK_ps = [ps.tile([1, 512], F32, tag="ps", name=f"ssqK{i}")
                           for i in range(2)]
                ssqQ_ps = ps.tile([1, 512], F32, tag="ps", name="ssqQ")
                ksq_t = {}

                def emit_ssqK(m):
                    ksq = ksq_t.pop(m)
                    for nh in range(2):
                        nc.tensor.matmul(
                            ssqK_ps[nh][:], sel[:, 0:1],
                            ksq[:, nh * 512:(nh + 1) * 512],
                            start=(m == 0), stop=(m == KT - 1),
                            skip_group_check=True)

                for m in range(KT):
                    for nh in range(2):
                        acc = ps.tile([128, 512], F32, tag="ps")
                        for kt in range(KT):
                            nc.tensor.matmul(
                                acc[:], wsl(wk, kt, m * 128, 128),
                                xsl(kt, nh * 512, 512),
                                start=(kt == 0), stop=(kt == KT - 1))
                        nc.scalar.activation(
                            kh(m)[:, nh * 512:(nh + 1) * 512], acc[:],
                            AF.Identity, bias=prm(5, m))
                    ksq = sqp.tile([128, N], BF, tag="sq")
                    nc.scalar.activation(ksq[:], kh(m), AF.Square)
                    ksq_t[m] = ksq
                    if m >= 1:
                        emit_ssqK(m - 1)
                emit_ssqK(KT - 1)

                # ---- Q phase ----
                qsq_t = {}
                for m in range(KT):
                    acc = ps.tile([128, 512], F32, tag="ps")
                    for kt in range(KT):
                        nc.tensor.matmul(
                            acc[:], wsl(wq, kt, m * 128, 128),
                            xsl(kt, 0, TL),
                            start=(kt == 0), stop=(kt == KT - 1))
                    nc.scalar.activation(qh(m), acc[:], AF.Identity,
                                         bias=prm(4, m))
                    qsq = sqp.tile([128, TL], BF, tag="sqq")
                    nc.scalar.activation(qsq[:], qh(m), AF.Square)
                    qsq_t[m] = qsq
                    if m >= 1:
                        nc.tensor.matmul(ssqQ_ps[:], sel[:, 0:1],
                                         qsq_t.pop(m - 1)[:],
                                         start=(m - 1 == 0), stop=False,
                                         skip_group_check=True)
                nc.tensor.matmul(ssqQ_ps[:], sel[:, 0:1],
                                 qsq_t.pop(KT - 1)[:],
                                 start=False, stop=True,
                                 skip_group_check=True)

                # ---- stats chain (rows 0/32/64, runs during V phase) ----
                nc.vector.tensor_copy(slr(0, 0), sums_ps[0][0:1, :])
                nc.vector.tensor_copy(slr(32, 0), sums_ps[1][0:1, :])
                nc.vector.tensor_copy(slr(64, 0), sums_ps[2][0:1, :])
                nc.vector.tensor_copy(slr(0, 1), ssqK_ps[0][0:1, :])
                nc.vector.tensor_copy(slr(32, 1), ssqK_ps[1][0:1, :])
                nc.vector.tensor_copy(slr(64, 1), ssqQ_ps[0:1, :])
                nc.vector.tensor_scalar(sl(2), sl(0), 1.0 / C, bsum[0:65],
                                        OP.mult, OP.add)
                nc.vector.tensor_mul(sl(3), sl(2), sl(2))
                nc.vector.tensor_scalar(sl(1), sl(1), 1.0 / C, LN_EPS,
                                        OP.mult, OP.add)
                nc.vector.tensor_sub(sl(1), sl(1), sl(3))
                nc.vector.tensor_scalar_max(sl(1), sl(1), 1e-20)
                nc.scalar.activation(sl(1), sl(1), AF.Ln)
                nc.scalar.activation(stmm[0:65, 0:512], sl(1), AF.Exp,
                                     scale=-0.5)
                nc.vector.tensor_mul(stmm[0:65, 512:1024], sl(2),
                                     stmm[0:65, 0:512])

                # ---- V phase (PE) with stat broadcasts interleaved;
                #      eviction on ACT; normalize (DVE) overlaps ----
                def bcast(row, slot, dst_col):
                    bc_ps = ps.tile([128, 512], F32, tag="ps")
                    nc.tensor.matmul(bc_ps[:], ones_big[row:row + 1, :],
                                     smmr(row, slot), start=True, stop=True,
                                     tile_position=(row, 0))
                    nc.vector.tensor_copy(rstdb[:, dst_col:dst_col + 512],
                                          bc_ps[:])

                def norm_k(m):
                    t = ntp.tile([128, N], BF, tag="nt")
                    nc.vector.tensor_mul(t[:], kh(m), rstdb[:, 0:N])
                    nc.vector.tensor_sub(t[:], t[:], rstdb[:, N:2 * N])
                    nc.vector.tensor_scalar(kh(m), t[:], prm(2, m),
                                            prm(3, m), OP.mult, OP.add)

                def norm_q(m):
                    tq = ntp.tile([128, TL], BF, tag="ntq")
                    nc.vector.tensor_mul(tq[:], qh(m),
                                         rstdb[:, 2 * N:2 * N + TL])
                    nc.vector.tensor_sub(
                        tq[:], tq[:], rstdb[:, 2 * N + TL:2 * N + 2 * TL])
                    nc.vector.tensor_scalar(qh(m), tq[:], prm(0, m),
                                            prm(1, m), OP.mult, OP.add)

                def emit_qz(kth):
                    qz = qzs[kth % 2]
                    nc.vector.tensor_copy(qz[0:64, 0:512],
                                          qhat[0:64,
                                               kth * TL:(kth + 1) * TL])
                    nc.vector.tensor_copy(qz[64:128, 512:1024],
                                          qhat[64:128,
                                               kth * TL:(kth + 1) * TL])

                ndone = 0
                for nh in range(2):
                    for mt in range(KT):
                        acc = ps.tile([128, 512], F32, tag="ps")
                        for kt in range(KT):
                            nc.tensor.matmul(
                                acc[:], xsl(kt, mt * 128, 128),
                                wsl(wv, kt, nh * 512, 512),
                                start=(kt == 0), stop=(kt == KT - 1))
                        dst = vv[:, mt, nh * 8:(nh + 1) * 8, 0:64]
                        nc.scalar.activation(
                            dst, acc[:].rearrange("p (h e) -> p h e", h=8),
                            AF.Copy)
                        step = nh * KT + mt
                        if step == 4:
                            bcast(0, 0, 0)            # K nh0 rstd
                            bcast(32, 0, 512)         # K nh1 rstd
                            bcast(0, 1, N)            # K nh0 mu*rstd
                            bcast(32, 1, N + 512)     # K nh1 mu*rstd
                            bcast(64, 0, 2 * N)       # Q rstd
                            bcast(64, 1, 2 * N + TL)  # Q mu*rstd
                        elif step >= 5 and ndone < KT:
                            norm_k(ndone)
                            norm_q(ndone)
                            if ndone == 0:
                                emit_qz(0)
                            ndone += 1
                for m in range(ndone, KT):
                    norm_k(m)
                    norm_q(m)
                    if m == 0:
                        emit_qz(0)

            # ---------- attention (software-pipelined across head pairs) ----
            with tc.tile_pool(name="sps", bufs=3, space="PSUM") as sps, \
                 tc.tile_pool(name="ops", bufs=2, space="PSUM") as ops, \
                 tc.tile_pool(name="pp", bufs=3) as ppool, \
                 tc.tile_pool(name="rcp", bufs=2) as rcp:
                s_tiles = {}

                def emit_S(kth, tt):
                    qz = qzs[kth % 2]
                    ksl = khat[:, kth * N + tt * 128: kth * N + (tt + 1) * 128]
                    s = sps.tile([128, 1024], F32, tag="s",
                                 name=f"s{kth}_{tt}")
                    nc.tensor.matmul(s[:, 0:512], ksl, qz[:, 0:512],
                                     start=True, stop=True)
                    nc.tensor.matmul(s[:, 512:1024], ksl, qz[:, 512:1024],
                                     start=True, stop=True)
                    s_tiles[(kth, tt)] = s

                def den_start(kth, o_psA, o_psB, den_first=False):
                    # cheap per-head-pair den capture, right after PV stop
                    dc = slice((kth % 2) * 512, (kth % 2) * 512 + 512)
                    ksl512 = slice(kth * 512, (kth + 1) * 512)

                    def dens():
                        nc.vector.tensor_copy(dden[0:1, dc], o_psA[64:65, :])
                        nc.vector.tensor_copy(dden[32:33, dc],
                                              o_psB[64:65, :])
                        nc.vector.reciprocal_approx_fast(
                            out=drecip[0:33, dc], in_=dden[0:33, dc])
                        nc.vector.tensor_copy(rcb[0:33, :], drecip[0:33, dc])

                    rcb = rcp.tile([64, 512], BF, tag="rcb")
                    if den_first:
                        dens()
                    nc.vector.tensor_copy(osb[0:64, ksl512], o_psA[0:64, :])
                    nc.vector.tensor_copy(osb[64:128, ksl512],
                                          o_psB[0:64, :])
                    if not den_first:
                        dens()
                    return rcb

                def den_finish(kth, rcb, pool=None):
                    # PE broadcast + DVE scale; deferred so it never blocks
                    # the next head-pair's S/PV matmuls
                    ksl512 = slice(kth * 512, (kth + 1) * 512)
                    bc_s = (pool or sps).tile([128, 1024], F32, tag="s",
                                              name=f"bc{kth}")
                    nc.tensor.matmul(bc_s[:, 0:512], ones_big[0:1, :],
                                     rcb[0:1, :], start=True, stop=True)
                    nc.tensor.matmul(bc_s[:, 512:1024], ones_big[32:33, :],
                                     rcb[32:33, :], start=True, stop=True,
                                     tile_position=(32, 0))
                    nc.vector.tensor_mul(osb[0:64, ksl512],
                                         osb[0:64, ksl512],
                                         bc_s[0:64, 0:512])
                    nc.vector.tensor_mul(osb[64:128, ksl512],
                                         osb[64:128, ksl512],
                                         bc_s[64:128, 512:1024])

                emit_S(0, 0)
                emit_S(0, 1)
                pending_den = None
                for kth in range(KT):
                    hA, hB = 2 * kth, 2 * kth + 1
                    o_psA = ops.tile([128, 512], F32, tag="o",
                                     name=f"oA{kth}")
                    o_psB = ops.tile([128, 512], F32, tag="o",
                                     name=f"oB{kth}")
                    for tt in range(8):
                        s = s_tiles.pop((kth, tt))
                        p = ppool.tile([128, 1024], BF, tag="p")
                        nc.scalar.activation(p[:, 0:512], s[:, 0:512],
                                             AF.Exp, scale=SCALE)
                        nc.scalar.activation(p[:, 512:1024], s[:, 512:1024],
                                             AF.Exp, scale=SCALE)
                        if tt == 2 and pending_den is not None:
                            den_finish(kth - 1, pending_den)
                            pending_den = None
                        nxt = tt + 2
                        if nxt < 8:
                            emit_S(kth, nxt)
                        elif kth + 1 < KT:
                            if nxt == 8:
                                emit_qz(kth + 1)
                                emit_S(kth + 1, 0)
                            elif nxt == 9:
                                emit_S(kth + 1, 1)
                        nc.tensor.matmul(o_psA[:], vf(tt, hA), p[:, 0:512],
                                         start=(tt == 0), stop=(tt == 7))
                        nc.tensor.matmul(o_psB[:], vf(tt, hB), p[:, 512:1024],
                                         start=(tt == 0), stop=(tt == 7))
                    if kth == KT - 1:
                        # capture kth=7 den early: den copies first so the
                        # reciprocal chain starts before the osb evictions
                        pending_den = den_start(kth, o_psA, o_psB,
                                                den_first=True)
                    else:
                        pending_den = den_start(kth, o_psA, o_psB)

                # ---- output projection, still inside the attention pool
                #      scope (no PSUM pool-transition barrier): m-outer,
                #      accumulators ride the s-tile pool rotation, each
                #      output tile evicts+DMAs while the next accumulates;
                #      the last head pair's den scale is woven in before
                #      its first use ----
                with tc.tile_pool(name="otp", bufs=3) as otp:
                    def pjmm(acc, m, kth):
                        nc.tensor.matmul(
                            acc,
                            wp[:, kth * C + m * 128: kth * C + (m + 1) * 128],
                            osb[:, kth * TL:(kth + 1) * TL],
                            start=(kth == 0), stop=(kth == KT - 1))

                    def pjout(acc, m):
                        ot = otp.tile([128, 512], F32, tag="ot")
                        nc.scalar.activation(ot[:], acc,
                                             AF.Identity, bias=prm(6, m))
                        nc.sync.dma_start(out_d[m * 128:(m + 1) * 128, :],
                                          ot[:])

                    # m=0 and m=1 chunks 0..6 both run before the last head
                    # pair's den broadcast so its DVE chain is fully covered
                    paccs = {}
                    for m in range(2):
                        accf = sps.tile([128, 1024], F32, tag="s",
                                        name=f"pacc{m}")
                        paccs[m] = accf[:, 0:512]
                        for kth in range(KT - 1):
                            pjmm(paccs[m], m, kth)
                    den_finish(KT - 1, pending_den)
                    for m in range(2):
                        pjmm(paccs[m], m, KT - 1)
                        pjout(paccs[m], m)
                    for m in range(2, KT):
                        accf = sps.tile([128, 1024], F32, tag="s",
                                        name=f"pacc{m}")
                        acc = accf[:, 0:512]
                        for kth in range(KT):
                            pjmm(acc, m, kth)
                        pjout(acc, m)

    nc.compile()
    return nc


def _get_compiled():
    global _COMPILED
    if _COMPILED is None:
        _COMPILED = _build()
    return _COMPILED


def _host_prep(x, qkv_w, qkv_b, qn_w, qn_b, kn_w, kn_b, proj_w, proj_b):
    import ml_dtypes
    BF = ml_dtypes.bfloat16
    qkv_w = np.asarray(qkv_w, np.float32)
    qkv_b = np.asarray(qkv_b, np.float32)
    proj_w = np.asarray(proj_w, np.float32)
    def slab(wT):  # [C, X] -> [128, KT*X] with row p, block kt
        X = wT.shape[1]
        return np.ascontiguousarray(
            wT.reshape(KT, 128, X).transpose(1, 0, 2).reshape(128, KT * X)
        ).astype(BF)

    qkvwT = np.ascontiguousarray(qkv_w.T)      # [C, 3C] f32
    wk_h = slab(qkvwT[:, C:2 * C])
    wq_h = slab(qkvwT[:, 0:C])
    wv_h = slab(qkvwT[:, 2 * C:3 * C])
    wp_h = slab(np.ascontiguousarray(proj_w.T))
    ws_q = qkv_w[0:C].sum(axis=0)
    ws_k = qkv_w[C:2 * C].sum(axis=0)
    wsums = np.zeros((128, 16), np.float32)
    for kt in range(8):
        wsums[:, kt * 2] = ws_q[kt * 128:(kt + 1) * 128]
        wsums[:, kt * 2 + 1] = ws_k[kt * 128:(kt + 1) * 128]
    wsums = wsums.astype(BF)
    bq = qkv_b[0:C].sum() / C
    bk = qkv_b[C:2 * C].sum() / C
    bsum3 = np.zeros((128, 1), np.float32)
    bsum3[0:64, 0] = bk      # rows 0 and 32 (K halves)
    bsum3[64:128, 0] = bq    # row 64 (Q)
    params = np.zeros((128, 56), np.float32)
    proj_b2 = np.asarray(proj_b, np.float32) + proj_w @ qkv_b[2 * C:3 * C]
    for g, vec in enumerate([qn_w, qn_b, kn_w, kn_b,
                             qkv_b[0:C], qkv_b[C:2 * C], proj_b2]):
        params[:, g * 8:(g + 1) * 8] = \
            np.asarray(vec, np.float32).reshape(8, 128).T

    in_maps = []
    for c in range(N_CORES):
        b, half = c // 2, c % 2
        xb = np.asarray(x[b], np.float32)
        xr = np.roll(xb, -half * TL, axis=0)   # local tokens -> rows [0,512)
        xTf = np.ascontiguousarray(xr.T)       # [C, N]
        xT = np.ascontiguousarray(
            xTf.reshape(KT, 128, N).transpose(1, 0, 2).reshape(128, KT * N)
        ).astype(BF)
        in_maps.append({
            "xT": xT, "wk": wk_h, "wq": wq_h, "wv": wv_h, "wp": wp_h,
            "wsums": wsums, "params": params, "bsum3": bsum3,
        })
    return in_maps


def _run(inputs, trace=False):
    from concourse.bass_utils import run_bass_kernel_spmd
    nc = _get_compiled()
    in_maps = _host_prep(**inputs)
    res = run_bass_kernel_spmd(nc, in_maps, core_ids=list(range(N_CORES)),
                               trace=trace)
    out = np.empty((B, N, C), np.float32)
    for c in range(N_CORES):
        b, half = c // 2, c % 2
        out[b, half * TL:(half + 1) * TL, :] = res.results[c]["out"].T
    return out, res


def kernel(**inputs):
    out, _ = _run(inputs, trace=False)
    return out
